# revision 87
# baseline (speedup 1.0000x reference)
"""DeepSeekMoE layer on 8 Trainium2 NeuronCores.

Strategy (two device phases):
  K1 — data-parallel over tokens (512 tokens/core): rmsnorm, router logits
       (PE-transposed fp32 matmul vs centroids^T), softmax, top-4 score mask.
  host — from the score mask, build per-expert token lists; gather + transpose
       activations to bf16 (pure layout work, no math beyond dtype cast).
  K2 — expert-parallel (4 routed experts/core, shared experts data-parallel on
       a 512-token slice): two matmuls per expert in bf16 with fp32 PSUM,
       fused bias+GELU on the scalar engine, gate-score weighting on device.
  host — scatter-add routed contributions, add shared + residual.
"""

import sys

sys.path.insert(0, "/opt/trn_rl_repo")

import math

import ml_dtypes
import numpy as np

import concourse.bass as bass
import concourse.mybir as mybir
import concourse.tile as tile
from concourse import bacc
from concourse.bass_utils import run_bass_kernel_spmd
from concourse.masks import make_identity

F32 = mybir.dt.float32
BF16 = mybir.dt.bfloat16
AF = mybir.ActivationFunctionType
ALU = mybir.AluOpType
BF16_NP = ml_dtypes.bfloat16

B, S, D = 2, 2048, 1024
H = 256          # expert hidden dim
NS, NR, TOPK = 2, 32, 4
EPS = 1e-6
NCORES = 8
T = B * S        # 4096 tokens
TSL = T // NCORES  # 512 tokens per core
EPC = NR // NCORES  # 4 routed experts per core


# --------------------------------------------------------------------------
# K1: rmsnorm + router (data-parallel)
# --------------------------------------------------------------------------
def build_k1():
    """Phase 1 (data-parallel, 512 tokens/core): rmsnorm (rms_w folded into
    weights host-side), router logits via PE-transposed raw x against
    rms_w-scaled centroids, fp32 softmax with the rmsnorm scale folded into
    the exp, and the top-4 score mask via DVE max8/match_replace."""
    nc = bacc.Bacc(None, target_bir_lowering=False)
    x = nc.dram_tensor("x", [TSL, D], F32, kind="ExternalInput")
    rms_w = nc.dram_tensor("rms_w", [1, D], F32, kind="ExternalInput")
    ct = nc.dram_tensor("ct", [D, NR], F32, kind="ExternalInput")  # centroids^T
    xn_out = nc.dram_tensor("xn", [TSL, D], F32, kind="ExternalOutput")
    aff_out = nc.dram_tensor("aff", [TSL, NR], F32, kind="ExternalOutput")
    mask_out = nc.dram_tensor("mask", [TSL, NR], F32, kind="ExternalOutput")
    NTI = TSL // 128  # token tiles per core

    with tile.TileContext(nc) as tc:
        with (
            tc.tile_pool(name="const", bufs=1) as cpool,
            tc.tile_pool(name="sh", bufs=1) as shpool,
            tc.tile_pool(name="work", bufs=5) as wpool,
            tc.tile_pool(name="small", bufs=8) as spool,
            tc.tile_pool(name="pst", bufs=5, space="PSUM") as pstpool,
            tc.tile_pool(name="pslg", bufs=3, space="PSUM") as pslgpool,
        ):
            ident = cpool.tile([128, 128], F32)
            make_identity(nc, ident[:])

            eps_t = cpool.tile([128, 1], F32)
            nc.vector.memset(eps_t[:], EPS)

            # centroids^T as 8 k-tiles of [128, 32], rms_w folded in
            ctiles = cpool.tile([128, 8, NR], F32)
            nc.sync.dma_start(ctiles[:], ct.rearrange("(k p) e -> p k e", p=128))
            rwkp = cpool.tile([128, 8], F32)
            nc.sync.dma_start(rwkp[:], rms_w.rearrange("o (k p) -> p (o k)", p=128))
            for k in range(8):
                nc.vector.tensor_scalar_mul(
                    ctiles[:, k, :], ctiles[:, k, :], rwkp[:, k : k + 1]
                )

            aff_all = shpool.tile([128, NTI, NR], F32)
            msk_all = shpool.tile([128, NTI, NR], F32)
            x_r = x.rearrange("(i p) d -> p i d", p=128)
            xn_r = xn_out.rearrange("(i p) d -> p i d", p=128)

            xts = []
            for i in range(NTI):
                xt = wpool.tile([128, D], F32, tag="xt")
                nc.sync.dma_start(xt[:], x_r[:, i, :])
                xts.append(xt)

            for i in range(NTI):
                xt = xts[i]
                xn = wpool.tile([128, D], F32, tag="xn")

                # rmsnorm (square+rowsum fused on DVE)
                xsq = wpool.tile([128, D], F32, tag="xsq")
                ssq = spool.tile([128, 1], F32, tag="ssq")
                nc.vector.scalar_tensor_tensor(
                    xsq[:], xt[:], 1.0, xt[:], op0=ALU.mult, op1=ALU.mult,
                    accum_out=ssq[:],
                )
                rms = spool.tile([128, 1], F32, tag="rms")
                nc.scalar.activation(
                    rms[:], ssq[:], AF.Sqrt, bias=eps_t[:], scale=1.0 / D
                )
                rstd = spool.tile([128, 1], F32, tag="rstd")
                nc.vector.reciprocal(rstd[:], rms[:])
                nc.vector.tensor_scalar_mul(xn[:], xt[:], rstd[:])
                nc.sync.dma_start(xn_r[:, i, :], xn[:])

                # transpose raw x (PE) for the router logits; drain on ACT/DVE
                xnt = wpool.tile([128, 8, 128], F32, tag="xnt")
                for g in range(4):
                    tps = pstpool.tile([128, 2, 128], F32, tag="tps")
                    for j in range(2):
                        k = g * 2 + j
                        nc.tensor.transpose(
                            tps[:, j, :], xt[:, k * 128 : (k + 1) * 128], ident[:]
                        )
                    if g % 2 == 0:
                        nc.vector.tensor_copy(xnt[:, g * 2 : g * 2 + 2, :], tps[:])
                    else:
                        nc.scalar.add(xnt[:, g * 2 : g * 2 + 2, :], tps[:], 0.0)

                lps = pslgpool.tile([128, NR], F32, tag="lps")
                for k in range(8):
                    nc.tensor.matmul(
                        lps[:],
                        xnt[:, k, :],
                        ctiles[:, k, :],
                        start=(k == 0),
                        stop=(k == 7),
                    )

                # softmax over 32 experts (rmsnorm scale folded into the exp)
                aff = aff_all[:, i, :]
                negmx = spool.tile([128, 1], F32, tag="negmx")
                nc.vector.reduce_max(
                    negmx[:], lps[:], axis=mybir.AxisListType.X, negate=True
                )
                nbias = spool.tile([128, 1], F32, tag="nbias")
                nc.vector.tensor_mul(nbias[:], negmx[:], rstd[:])
                sume = spool.tile([128, 1], F32, tag="sume")
                nc.scalar.activation(
                    aff, lps[:], AF.Exp, bias=nbias[:], scale=rstd[:],
                    accum_out=sume[:],
                )
                rsum = spool.tile([128, 1], F32, tag="rsum")
                nc.vector.reciprocal(rsum[:], sume[:])
                nc.vector.tensor_scalar_mul(aff, aff, rsum[:])

                # top-4 score mask: affinity where in top-4 else 0
                msk = msk_all[:, i, :]
                m8 = spool.tile([128, 8], F32, tag="m8")
                nc.vector.max(m8[:], aff)
                nc.vector.memset(m8[:, TOPK:], 0.0)
                zap = wpool.tile([128, NR], F32, tag="zap")
                nc.vector.match_replace(zap[:], m8[:], aff, 0.0)
                nc.vector.tensor_sub(msk, aff, zap[:])

            nc.sync.dma_start(
                aff_out.rearrange("(i p) e -> p i e", p=128), aff_all[:]
            )
            nc.sync.dma_start(
                mask_out.rearrange("(i p) e -> p i e", p=128), msk_all[:]
            )

    nc.compile()
    return nc


# --------------------------------------------------------------------------
# K2: expert MLPs (expert-parallel for routed, data-parallel for shared)
# --------------------------------------------------------------------------
def build_k2(caps):
    """Phase 2 (expert-parallel): 4 routed experts per core on gathered
    tokens + the 2 shared experts on this core's 512-token slice.
    caps[j] is the token capacity of expert slot j (experts are assigned to
    slots by descending load on the host, so slot capacities shrink).
    Emission is software-pipelined: each producer's MM1 groups are
    interleaved between the previous producer's MM2 groups so the PE
    streams continuously."""
    NT = sum(caps)
    off = [sum(caps[:j]) for j in range(EPC)]
    maxcap = max(caps)
    nc = bacc.Bacc(None, target_bir_lowering=False)
    xgt = nc.dram_tensor("xgt", [D, NT], BF16, kind="ExternalInput")
    xst = nc.dram_tensor("xst", [D, TSL], BF16, kind="ExternalInput")
    scores = nc.dram_tensor("scores", [1, NT], BF16, kind="ExternalInput")
    w1 = nc.dram_tensor("w1", [EPC, D, H], BF16, kind="ExternalInput")
    b1 = nc.dram_tensor("b1", [EPC, H], F32, kind="ExternalInput")
    w2 = nc.dram_tensor("w2", [EPC, H, D], BF16, kind="ExternalInput")
    b2 = nc.dram_tensor("b2", [EPC, D], F32, kind="ExternalInput")
    sw1 = nc.dram_tensor("sw1", [NS, D, H], BF16, kind="ExternalInput")
    sb1 = nc.dram_tensor("sb1", [NS, H], F32, kind="ExternalInput")
    sw2 = nc.dram_tensor("sw2", [NS, H, D], BF16, kind="ExternalInput")
    sb2 = nc.dram_tensor("sb2", [NS, D], F32, kind="ExternalInput")
    rout = nc.dram_tensor("rout", [D, NT], BF16, kind="ExternalOutput")
    shout = nc.dram_tensor("shout", [D, TSL], BF16, kind="ExternalOutput")

    def chunks_of(n):
        if n <= 512:
            return [(0, n)]
        # split >512 into near-equal halves (each <=512) instead of (512, tail)
        h = (n // 2 + 31) // 32 * 32
        return [(0, h), (h, n)]

    with tile.TileContext(nc) as tc:
        with (
            tc.tile_pool(name="const", bufs=1) as cpool,
            tc.tile_pool(name="work", bufs=5) as wpool,
            tc.tile_pool(name="out", bufs=8) as opool,
            tc.tile_pool(name="robuf", bufs=3) as ropool,
            tc.tile_pool(name="shbuf", bufs=1) as shpool,
            tc.tile_pool(name="psh", bufs=2, space="PSUM") as pshpool,
            tc.tile_pool(name="pso", bufs=6, space="PSUM") as psopool,
        ):
            # expert-0 operands first: PE can start after ~2MB of DMA
            xg = cpool.tile([128, 8, NT], BF16)
            w1t = cpool.tile([128, EPC, 8, H], BF16)
            w2t = cpool.tile([128, EPC, 2, D], BF16)
            xgt_r = xgt.rearrange("(k p) n -> p k n", p=128)
            w1_r = w1.rearrange("e (k p) h -> p e k h", p=128)
            w2_r = w2.rearrange("e (k p) d -> p e k d", p=128)
            xg = cpool.tile([128, 8, NT], BF16)
            w1t = cpool.tile([128, EPC, 8, H], BF16)
            w2t = cpool.tile([128, EPC, 2, D], BF16)
            xgt_r = xgt.rearrange("(k p) n -> p k n", p=128)
            w1_r = w1.rearrange("e (k p) h -> p e k h", p=128)
            w2_r = w2.rearrange("e (k p) d -> p e k d", p=128)
            b1t = cpool.tile([128, EPC, 2], F32)
            nc.sync.dma_start(b1t[:], b1.rearrange("e (m p) -> p e m", p=128))
            b2t = cpool.tile([128, EPC, 8], F32)
            nc.sync.dma_start(b2t[:], b2.rearrange("e (m p) -> p e m", p=128))
            sb1t = cpool.tile([128, NS, 2], F32)
            nc.sync.dma_start(sb1t[:], sb1.rearrange("e (m p) -> p e m", p=128))
            sb2t = cpool.tile([128, NS, 8], F32)
            nc.sync.dma_start(sb2t[:], sb2.rearrange("e (m p) -> p e m", p=128))
            # gate scores: one DMA, one broadcast to all partitions
            sc_row = cpool.tile([1, NT], BF16)
            nc.sync.dma_start(sc_row[:], scores[:, :])
            sc_all = cpool.tile([128, NT], BF16)
            nc.gpsimd.partition_broadcast(sc_all[:], sc_row[:])
            nc.sync.dma_start(w1t[:, 0:1, :, :], w1_r[:, 0:1, :, :])
            nc.sync.dma_start(xg[:, :, 0 : caps[0]], xgt_r[:, :, 0 : caps[0]])

            # shared-expert operands
            xs = cpool.tile([128, 8, TSL], BF16)
            nc.sync.dma_start(xs[:], xst.rearrange("(k p) n -> p k n", p=128))
            sw1t = cpool.tile([128, NS, 8, H], BF16)
            nc.sync.dma_start(sw1t[:], sw1.rearrange("e (k p) h -> p e k h", p=128))
            nc.sync.dma_start(w2t[:, 0:1, :, :], w2_r[:, 0:1, :, :])
            sw2t = cpool.tile([128, NS, 2, D], BF16)
            nc.sync.dma_start(sw2t[:], sw2.rearrange("e (k p) d -> p e k d", p=128))

            # PE warm-up: dummy matmuls during the initial DMA window so the
            # HAM clock-gate opens (2.4 GHz) before the first real matmul
            wu_w = cpool.tile([128, 128], BF16)
            nc.vector.memset(wu_w[:], 0.0)
            wu_r = cpool.tile([128, 512], BF16)
            nc.vector.memset(wu_r[:], 0.0)
            wu_ps = pshpool.tile([128, 512], F32, tag="ps_h")
            for _ in range(9):
                nc.tensor.matmul(
                    wu_ps[:], wu_w[:], wu_r[:], start=True, stop=True
                )

            # remaining routed operands: per-expert DMAs so expert e can
            # start as soon as its own slices have landed
            for le in range(1, EPC):
                nc.sync.dma_start(
                    w1t[:, le : le + 1, :, :], w1_r[:, le : le + 1, :, :]
                )
                nc.sync.dma_start(
                    xg[:, :, off[le] : off[le] + caps[le]],
                    xgt_r[:, :, off[le] : off[le] + caps[le]],
                )
                nc.sync.dma_start(
                    w2t[:, le : le + 1, :, :], w2_r[:, le : le + 1, :, :]
                )
            def routed_mm1_unit(le, h_sb, m, a, b):
                def emit():
                    ps = pshpool.tile([128, b - a], F32, tag="ps_h")
                    for k in range(8):
                        nc.tensor.matmul(
                            ps[:],
                            w1t[:, le, k, m * 128 : (m + 1) * 128],
                            xg[:, k, off[le] + a : off[le] + b],
                            start=(k == 0),
                            stop=(k == 7),
                        )
                    nc.vector.tensor_scalar_add(
                        h_sb[:, m, a:b], ps[:], b1t[:, le, m : m + 1]
                    )
                return emit

            def routed_mm2_unit(le, h_sb, ro_all, m2, a, b):
                def emit():
                    ps = psopool.tile([128, b - a], F32, tag="ps_o")
                    for k2 in range(2):
                        nc.tensor.matmul(
                            ps[:],
                            w2t[:, le, k2, m2 * 128 : (m2 + 1) * 128],
                            h_sb[:, k2, a:b],
                            start=(k2 == 0),
                            stop=(k2 == 1),
                        )
                    og = opool.tile([128, 512], BF16, tag="og")
                    nc.scalar.activation(
                        og[:, : b - a], ps[:], AF.Gelu,
                        bias=b2t[:, le, m2 : m2 + 1], scale=1.0,
                    )
                    nc.vector.tensor_mul(
                        ro_all[:, m2, a:b], og[:, : b - a],
                        sc_all[:, off[le] + a : off[le] + b],
                    )
                return emit

            def shared_mm1_unit(s, h_sb, m):
                def emit():
                    ps = pshpool.tile([128, TSL], F32, tag="ps_h")
                    for k in range(8):
                        nc.tensor.matmul(
                            ps[:],
                            sw1t[:, s, k, m * 128 : (m + 1) * 128],
                            xs[:, k, :],
                            start=(k == 0),
                            stop=(k == 7),
                        )
                    nc.vector.tensor_scalar_add(
                        h_sb[:, m, :TSL], ps[:], sb1t[:, s, m : m + 1]
                    )
                return emit

            def shared_mm2_unit(s, h_sb, m2):
                def emit():
                    ps = psopool.tile([128, TSL], F32, tag="ps_o")
                    for k2 in range(2):
                        nc.tensor.matmul(
                            ps[:],
                            sw2t[:, s, k2, m2 * 128 : (m2 + 1) * 128],
                            h_sb[:, k2, :TSL],
                            start=(k2 == 0),
                            stop=(k2 == 1),
                        )
                    if s == 0:
                        nc.scalar.activation(
                            sh_all[:, m2, :], ps[:], AF.Gelu,
                            bias=sb2t[:, s, m2 : m2 + 1], scale=1.0,
                        )
                    else:
                        og = opool.tile([128, 512], BF16, tag="og")
                        nc.scalar.activation(
                            og[:, :TSL], ps[:], AF.Gelu,
                            bias=sb2t[:, s, m2 : m2 + 1], scale=1.0,
                        )
                        nc.vector.tensor_add(
                            sh_all[:, m2, :], sh_all[:, m2, :], og[:, :TSL]
                        )
                return emit

            sh_all = shpool.tile([128, 8, TSL], BF16, tag="sh_all")

            producers = []
            routed_prods = []
            for s in range(1):
                h_sb = wpool.tile([128, 2, maxcap], BF16, tag="h_sb")
                mm1 = [shared_mm1_unit(s, h_sb, m) for m in range(2)]
                mm2 = [shared_mm2_unit(s, h_sb, m2) for m2 in range(8)]

                def shflush(s=s):
                    pass
                producers.append((mm1, mm2, shflush))
            for le in range(EPC):
                h_sb = wpool.tile([128, 2, caps[le]], BF16, tag="h_sb")
                ro_all = ropool.tile([128, 8, caps[le]], BF16, tag="ro_all")
                mm1 = [
                    routed_mm1_unit(le, h_sb, m, a, b)
                    for m in range(2)
                    for a, b in chunks_of(caps[le])
                ]
                mm2 = [
                    routed_mm2_unit(le, h_sb, ro_all, m2, a, b)
                    for m2 in range(8)
                    for a, b in chunks_of(caps[le])
                ]

                def rohalf(ro_all=ro_all, le=le, lo=0, hi=4):
                    def emit():
                        nc.sync.dma_start(
                            rout.rearrange("(m p) n -> p m n", p=128)[
                                :, lo:hi, off[le] : off[le] + caps[le]
                            ],
                            ro_all[:, lo:hi, :],
                        )
                    return emit

                half = len(mm2) // 2
                mm2.insert(half, rohalf(ro_all, le, 0, 4))
                mm2.append(rohalf(ro_all, le, 4, 8))

                def roflush():
                    pass
                routed_prods.append((mm1, mm2, roflush))
            producers = [routed_prods[0], producers[0]] + routed_prods[1:]
            for s in range(1, NS):
                h_sb = wpool.tile([128, 2, maxcap], BF16, tag="h_sb")
                mm1 = [shared_mm1_unit(s, h_sb, m) for m in range(2)]
                mm2 = [shared_mm2_unit(s, h_sb, m2) for m2 in range(8)]

                def shhalf(lo, hi):
                    def emit():
                        nc.sync.dma_start(
                            shout.rearrange("(m p) n -> p m n", p=128)[:, lo:hi, :],
                            sh_all[:, lo:hi, :],
                        )
                    return emit

                mm2.insert(4, shhalf(0, 4))
                mm2.append(shhalf(4, 8))

                def shflush2():
                    pass
                producers.append((mm1, mm2, shflush2))

            # pipelined emission
            prev_mm2 = None
            prev_flush = None
            for mm1, mm2, fl in producers:
                if prev_mm2 is None:
                    for u in mm1:
                        u()
                else:
                    n1, n2 = len(mm1), len(prev_mm2)
                    j = 0
                    for i, u in enumerate(mm1):
                        u()
                        take = (n2 * (i + 1)) // n1 - j
                        for u2 in prev_mm2[j : j + take]:
                            u2()
                        j += take
                    for u2 in prev_mm2[j:]:
                        u2()
                    prev_flush()
                prev_mm2, prev_flush = mm2, fl
            for u2 in prev_mm2:
                u2()
            prev_flush()

    nc.compile()
    return nc


# --------------------------------------------------------------------------
# host orchestration
# --------------------------------------------------------------------------
_cache = {}
TRACE = False
LAST_RESULTS = {}


def _get_k1():
    if "k1" not in _cache:
        _cache["k1"] = build_k1()
    return _cache["k1"]


def _get_k2(caps):
    key = ("k2", caps)
    if key not in _cache:
        _cache[key] = build_k2(caps)
    return _cache[key]


def kernel(x, rms_w, centroids, sW1, sb1, sW2, sb2, rW1, rb1, rW2, rb2):
    x = np.asarray(x, dtype=np.float32)
    rms_w = np.asarray(rms_w, dtype=np.float32)
    centroids = np.asarray(centroids, dtype=np.float32)
    sW1 = np.asarray(sW1, dtype=np.float32)
    sb1 = np.asarray(sb1, dtype=np.float32)
    sW2 = np.asarray(sW2, dtype=np.float32)
    sb2 = np.asarray(sb2, dtype=np.float32)
    rW1 = np.asarray(rW1, dtype=np.float32)
    rb1 = np.asarray(rb1, dtype=np.float32)
    rW2 = np.asarray(rW2, dtype=np.float32)
    rb2 = np.asarray(rb2, dtype=np.float32)

    xf = np.ascontiguousarray(x.reshape(T, D))
    ct = np.ascontiguousarray(centroids.T)
    # device xn excludes the rms_w gain; fold it into all consumers of xn
    rW1 = rW1 * rms_w[None, :, None]
    sW1 = sW1 * rms_w[None, :, None]

    # ---- phase 1 ----
    nc1 = _get_k1()
    in_maps = [
        {
            "x": xf[c * TSL : (c + 1) * TSL],
            "rms_w": rms_w.reshape(1, D),
            "ct": ct,
        }
        for c in range(NCORES)
    ]
    r1 = run_bass_kernel_spmd(
        nc1, in_maps, core_ids=list(range(NCORES)), trace=TRACE
    )
    LAST_RESULTS["k1"] = r1
    xn = np.concatenate([r1.results[c]["xn"] for c in range(NCORES)], axis=0)
    aff = np.concatenate([r1.results[c]["aff"] for c in range(NCORES)], axis=0)
    mask = np.concatenate([r1.results[c]["mask"] for c in range(NCORES)], axis=0)

    # ---- host dispatch: per-expert token lists from the score mask ----
    tok_idx = []
    tok_scores = []
    for e in range(NR):
        nz = np.nonzero(mask[:, e])[0]
        tok_idx.append(nz)
        tok_scores.append(mask[nz, e])
    counts = np.array([len(ix) for ix in tok_idx])
    # assign experts to (core, slot): slot j takes the j-th group of 8 in
    # descending-load order, so each slot's capacity hugs its max load
    order = np.argsort(-counts, kind="stable")
    slot_expert = [[int(order[j * NCORES + c]) for j in range(EPC)] for c in range(NCORES)]
    caps = tuple(
        max(128, int(math.ceil(counts[order[j * NCORES]] / 32.0)) * 32)
        for j in range(EPC)
    )
    off = [sum(caps[:j]) for j in range(EPC)]
    NT = sum(caps)

    xn_bf = xn.astype(BF16_NP)
    in_maps2 = []
    for c in range(NCORES):
        xgt = np.zeros((D, NT), dtype=BF16_NP)
        sc = np.zeros((1, NT), dtype=BF16_NP)
        for le in range(EPC):
            e = slot_expert[c][le]
            n_e = len(tok_idx[e])
            xgt[:, off[le] : off[le] + n_e] = xn_bf[tok_idx[e]].T
            sc[0, off[le] : off[le] + n_e] = tok_scores[e]
        xst = np.ascontiguousarray(xn_bf[c * TSL : (c + 1) * TSL].T)
        es = slot_expert[c]
        in_maps2.append(
            {
                "xgt": xgt,
                "xst": xst,
                "scores": sc,
                "w1": rW1[es].astype(BF16_NP),
                "b1": rb1[es],
                "w2": rW2[es].astype(BF16_NP),
                "b2": rb2[es],
                "sw1": sW1.astype(BF16_NP),
                "sb1": sb1,
                "sw2": sW2.astype(BF16_NP),
                "sb2": sb2,
            }
        )

    # ---- phase 2 ----
    nc2 = _get_k2(caps)
    r2 = run_bass_kernel_spmd(
        nc2, in_maps2, core_ids=list(range(NCORES)), trace=TRACE
    )
    LAST_RESULTS["k2"] = r2

    # ---- host combine: y = xn + shared + scatter(routed) ----
    y = xn * rms_w[None, :]
    for c in range(NCORES):
        y[c * TSL : (c + 1) * TSL] += r2.results[c]["shout"].astype(np.float32).T
        routed = r2.results[c]["rout"].astype(np.float32)
        for le in range(EPC):
            e = slot_expert[c][le]
            n_e = len(tok_idx[e])
            if n_e:
                y[tok_idx[e]] += routed[:, off[le] : off[le] + n_e].T

    return y.reshape(B, S, D), aff
